# revision 32
# baseline (speedup 1.0000x reference)
"""Two-layer GAT on 8 Trainium2 NeuronCores — old gather structure + bf16.

Same design as the original baseline (dense dst-window edge columns, int32
indirect gathers, one-hot scatter with PE-transpose a_dst expansion) but with
bf16 tables, gathers, one-hots, and matmuls: ~2x less gather traffic and ~4x
faster PE ops. Scores/softmax stay fp32. Epsilon added to softmax denominators
(reference does the same) so empty pad rows yield finite garbage instead of
NaN that poisons layer-2 matmuls.
"""

import numpy as np

# ---- problem constants (hardcoded per harness contract) ----
N = 100000
E = 1600000
IN = 128
HID = 16
HEADS = 8
OUT = 64
NEG = 0.2
NC = 8
NLOC = N // NC          # 12500
WIN = 128
NWIN = (NLOC + WIN - 1) // WIN   # 98
LAST_ROWS = NLOC - (NWIN - 1) * WIN  # 84
C1 = HEADS * HID        # 128
ROW1 = C1 + 2 * HEADS   # 144 = z(128) | a_src(8) | a_dst(8)
ROW2 = OUT + 2          # 66  = h2(64) | a_src2(1) | a_dst2(1)
BATCH = 16              # edge groups per batched compute slab
PHA_B = 3               # Phase A node tiles per psum bank (3*144=432 <= 512)


def _preprocess(edge_index):
    """Per-core edge arrays in partition-major layout + shared group counts."""
    src = np.concatenate([np.asarray(edge_index[0]), np.arange(N)]).astype(np.int64)
    dst = np.concatenate([np.asarray(edge_index[1]), np.arange(N)]).astype(np.int64)
    core = dst // NLOC
    per_core = []
    cnts = np.zeros((NC, NWIN), dtype=np.int64)
    for k in range(NC):
        m = core == k
        s, d = src[m], dst[m] - k * NLOC
        o = np.argsort(d, kind="stable")
        s, d = s[o], d[o]
        per_core.append((s, d))
        cnts[k] = np.bincount(d // WIN, minlength=NWIN)
    ngroups = np.maximum(1, ((cnts + 127) // 128).max(axis=0))  # shared, >=1
    G = int(ngroups.sum())
    gstart = np.concatenate([[0], np.cumsum(ngroups)])
    srcsT = np.zeros((NC, 128, G), dtype=np.int32)
    dstwT = np.full((NC, 128, G), 999.0, dtype=np.float32)
    for k in range(NC):
        s, d = per_core[k]
        w = d // WIN
        ws = np.searchsorted(w, np.arange(NWIN))
        we = np.searchsorted(w, np.arange(NWIN), side="right")
        fs = np.zeros(G * 128, dtype=np.int64)
        fw = np.full(G * 128, 999.0, dtype=np.float32)
        for wi in range(NWIN):
            cnt = we[wi] - ws[wi]
            a = gstart[wi] * 128
            fs[a:a + cnt] = s[ws[wi]:we[wi]]
            fw[a:a + cnt] = (d[ws[wi]:we[wi]] - wi * WIN).astype(np.float32)
        srcsT[k] = fs.reshape(G, 128).T.astype(np.int32)
        dstwT[k] = fw.reshape(G, 128).T
    return srcsT, dstwT, ngroups.tolist(), G


def _pack_weights(W1, att_src1, att_dst1, W2, att_src2, att_dst2):
    import ml_dtypes
    W1 = np.asarray(W1, np.float32)
    W2 = np.asarray(W2, np.float32)
    A1s = np.zeros((C1, HEADS), np.float32)
    A1d = np.zeros((C1, HEADS), np.float32)
    for h in range(HEADS):
        A1s[h * HID:(h + 1) * HID, h] = np.asarray(att_src1, np.float32)[h]
        A1d[h * HID:(h + 1) * HID, h] = np.asarray(att_dst1, np.float32)[h]
    W1ext = np.concatenate([W1, W1 @ A1s, W1 @ A1d], axis=1)   # [128, 144]
    W2ext = np.concatenate(
        [W2, W2 @ np.asarray(att_src2, np.float32).T,
         W2 @ np.asarray(att_dst2, np.float32).T], axis=1)     # [128, 66]
    return (np.ascontiguousarray(W1ext).astype(ml_dtypes.bfloat16),
            np.ascontiguousarray(W2ext).astype(ml_dtypes.bfloat16))


def _build_nc(ngroups, G):
    import concourse.bass as bass
    import concourse.bacc as bacc
    import concourse.mybir as mybir
    import concourse.tile as tile

    dt = mybir.dt
    AF = mybir.ActivationFunctionType
    OP = mybir.AluOpType
    nc = bacc.Bacc("TRN2", target_bir_lowering=False, debug=False, num_devices=NC)

    xT = nc.dram_tensor("xT", [IN, N], dt.bfloat16, kind="ExternalInput")
    W1e = nc.dram_tensor("W1e", [IN, ROW1], dt.bfloat16, kind="ExternalInput")
    W2e = nc.dram_tensor("W2e", [C1, ROW2], dt.bfloat16, kind="ExternalInput")
    iota2d = nc.dram_tensor("iota2d", [128, 128], dt.bfloat16, kind="ExternalInput")
    ident = nc.dram_tensor("ident", [128, 128], dt.bfloat16, kind="ExternalInput")
    b1_2d = nc.dram_tensor("b1_2d", [128, C1], dt.float32, kind="ExternalInput")
    b2_2d = nc.dram_tensor("b2_2d", [128, OUT], dt.float32, kind="ExternalInput")
    srcsT = nc.dram_tensor("srcsT", [128, G], dt.int32, kind="ExternalInput")
    dstwT = nc.dram_tensor("dstwT", [128, G], dt.float32, kind="ExternalInput")
    xownT = nc.dram_tensor("xownT", [IN, NWIN * 128], dt.bfloat16, kind="ExternalInput")
    out = nc.dram_tensor("out", [NLOC, OUT], dt.float32, kind="ExternalOutput")

    hext1 = nc.dram_tensor("hext1", [N, ROW1], dt.bfloat16)
    adstloc = nc.dram_tensor("adstloc", [NWIN * 128, HEADS], dt.bfloat16)
    h2own = nc.dram_tensor("h2own", [NLOC, ROW2], dt.bfloat16)
    h2full = nc.dram_tensor("h2full", [N, ROW2], dt.bfloat16, addr_space="Shared")

    gstart = np.concatenate([[0], np.cumsum(ngroups)]).astype(int)

    with tile.TileContext(nc) as tc:
        with tc.tile_pool(name="const", bufs=1) as cb:
            w1e_t = cb.tile([IN, ROW1], dt.bfloat16)
            nc.sync.dma_start(out=w1e_t[:], in_=W1e[:, :])
            w2e_t = cb.tile([C1, ROW2], dt.bfloat16)
            nc.sync.dma_start(out=w2e_t[:], in_=W2e[:, :])
            iota_t = cb.tile([128, 128], dt.bfloat16)
            nc.sync.dma_start(out=iota_t[:], in_=iota2d[:, :])
            ident_t = cb.tile([128, 128], dt.bfloat16)
            nc.sync.dma_start(out=ident_t[:], in_=ident[:, :])
            b1_t = cb.tile([128, C1], dt.float32)
            nc.sync.dma_start(out=b1_t[:], in_=b1_2d[:, :])
            b2_t = cb.tile([128, OUT], dt.float32)
            nc.sync.dma_start(out=b2_t[:], in_=b2_2d[:, :])
            tc.strict_bb_all_engine_barrier()

            # ---------------- Phase A: hext1 = x @ W1ext (replicated) --------
            with (
                tc.tile_pool(name="pha_sb", bufs=3) as sa,
                tc.tile_pool(name="pha_ps", bufs=2, space="PSUM") as pa,
            ):
                ntile = (N + 127) // 128  # 782, last has 32 rows
                t = 0
                while t < ntile:
                    nb = min(PHA_B, ntile - t)
                    r0 = t * 128
                    rows = min(nb * 128, N - r0)
                    xt = sa.tile([IN, nb * 128], dt.bfloat16, tag="xt")
                    nc.sync.dma_start(out=xt[:, :rows], in_=xT[:, r0:r0 + rows])
                    psA = pa.tile([128, nb * ROW1], dt.float32, tag="psA")
                    for b in range(nb):
                        rr = min(128, N - (t + b) * 128)
                        nc.tensor.matmul(
                            out=psA[:rr, b * ROW1:(b + 1) * ROW1],
                            lhsT=xt[:, b * 128:b * 128 + rr],
                            rhs=w1e_t[:], start=True, stop=True)
                    zs = sa.tile([128, nb * ROW1], dt.bfloat16, tag="zs")
                    if rows % 128 == 0:
                        nc.vector.tensor_copy(out=zs[:], in_=psA[:])
                        nc.sync.dma_start(
                            out=hext1[r0:r0 + rows, :].rearrange(
                                "(b p) f -> p b f", p=128),
                            in_=zs[:].rearrange("p (b f) -> p b f", b=nb))
                    else:
                        for b in range(nb):
                            rr = min(128, N - (t + b) * 128)
                            nc.vector.tensor_copy(
                                out=zs[:rr, b * ROW1:(b + 1) * ROW1],
                                in_=psA[:rr, b * ROW1:(b + 1) * ROW1])
                            nc.sync.dma_start(
                                out=hext1[(t + b) * 128:(t + b) * 128 + rr, :],
                                in_=zs[:rr, b * ROW1:(b + 1) * ROW1])
                    t += nb
                # Phase A2: own-shard a_dst table (window-padded, core-local)
                for w in range(NWIN):
                    xo = sa.tile([IN, 128], dt.bfloat16, tag="xo")
                    nc.sync.dma_start(out=xo[:], in_=xownT[:, w * 128:(w + 1) * 128])
                    psA2 = pa.tile([128, HEADS], dt.float32, tag="psA2")
                    nc.tensor.matmul(out=psA2[:], lhsT=xo[:],
                                     rhs=w1e_t[:, C1 + HEADS:ROW1], start=True, stop=True)
                    a2s = sa.tile([128, HEADS], dt.bfloat16, tag="a2s")
                    nc.vector.tensor_copy(out=a2s[:], in_=psA2[:])
                    nc.sync.dma_start(out=adstloc[w * 128:(w + 1) * 128, :], in_=a2s[:])
            tc.strict_bb_all_engine_barrier()

            # ---------------- edge aggregation pipeline ----------------------
            def edge_layer(table_ap, feat, nh, adw_src, flush):
                S = feat + nh
                with (
                    tc.tile_pool(name="eb_sb", bufs=3) as sb,
                    tc.tile_pool(name="eb_idx", bufs=2) as sx,
                    tc.tile_pool(name="eb_ps", bufs=2, space="PSUM") as pw,
                    tc.tile_pool(name="eb_pot", bufs=2, space="PSUM") as pot,
                    tc.tile_pool(name="eb_pad", bufs=2, space="PSUM") as pad,
                    tc.tile_pool(name="eb_ps2", bufs=1, space="PSUM") as p2,
                ):
                    for w in range(NWIN):
                        g0, g1 = int(gstart[w]), int(gstart[w + 1])
                        ng = g1 - g0
                        src_t = sx.tile([128, ng], dt.int32, tag="src")
                        dw_t = sx.tile([128, ng], dt.float32, tag="dw")
                        nc.sync.dma_start(out=src_t[:], in_=srcsT[:, g0:g1])
                        nc.sync.dma_start(out=dw_t[:], in_=dstwT[:, g0:g1])
                        adw_ap, adw_rows = adw_src(w)
                        adw_t = sx.tile([128, nh], dt.bfloat16, tag="adw")
                        if adw_rows < 128:
                            nc.gpsimd.memset(adw_t[:], 0.0)
                        nc.sync.dma_start(out=adw_t[:adw_rows, :], in_=adw_ap)
                        psW = pw.tile([128, S], dt.float32, tag="psW")
                        j = 0
                        first = True
                        while j < ng:
                            nb = min(BATCH, ng - j)
                            hx = sb.tile([128, BATCH * S], dt.bfloat16, tag="hx")
                            ad = sb.tile([128, BATCH * nh], dt.float32, tag="ad")
                            for b in range(nb):
                                nc.gpsimd.indirect_dma_start(
                                    out=hx[:, b * S:(b + 1) * S],
                                    out_offset=None, in_=table_ap,
                                    in_offset=bass.IndirectOffsetOnAxis(
                                        ap=src_t[:, j + b:j + b + 1], axis=0))
                            Ot = sb.tile([128, BATCH * 128], dt.bfloat16, tag="Ot")
                            for b in range(nb):
                                nc.vector.tensor_scalar(
                                    out=Ot[:, b * 128:(b + 1) * 128], in0=iota_t[:],
                                    scalar1=dw_t[:, j + b:j + b + 1], scalar2=None,
                                    op0=OP.is_equal)
                            # a_dst_e = O @ a_dstW  (transpose O on PE, then matmul)
                            for b in range(nb):
                                psOT = pot.tile([128, 128], dt.bfloat16, tag="psOT")
                                nc.tensor.transpose(
                                    out=psOT[:], in_=Ot[:, b * 128:(b + 1) * 128],
                                    identity=ident_t[:])
                                ot_sb = sb.tile([128, 128], dt.bfloat16, tag="otsb")
                                nc.scalar.copy(out=ot_sb[:], in_=psOT[:])
                                psAD = pad.tile([128, nh], dt.float32, tag="psAD")
                                nc.tensor.matmul(out=psAD[:], lhsT=ot_sb[:],
                                                 rhs=adw_t[:], start=True, stop=True)
                                nc.scalar.copy(out=ad[:, b * nh:(b + 1) * nh], in_=psAD[:])
                            # e = a_src + a_dst ; w = exp(max(e, 0.2e))
                            ev = sb.tile([128, BATCH * nh], dt.float32, tag="ev")
                            asrc_v = hx[:].rearrange("p (b f) -> p b f", b=BATCH)[:, :nb, feat:S]
                            nc.vector.tensor_tensor(
                                out=ev[:, :nb * nh].rearrange("p (b h) -> p b h", b=nb),
                                in0=asrc_v, in1=ad[:, :nb * nh].rearrange(
                                    "p (b h) -> p b h", b=nb), op=OP.add)
                            sc = sb.tile([128, BATCH * nh], dt.float32, tag="sc")
                            nc.scalar.mul(out=sc[:, :nb * nh], in_=ev[:, :nb * nh], mul=NEG)
                            w8 = sb.tile([128, BATCH * nh], dt.float32, tag="w8")
                            nc.vector.tensor_tensor(out=w8[:, :nb * nh], in0=ev[:, :nb * nh],
                                                    in1=sc[:, :nb * nh], op=OP.max)
                            nc.scalar.activation(out=w8[:, :nb * nh], in_=w8[:, :nb * nh],
                                                 func=AF.Exp)
                            # weighted rhs
                            rhs = sb.tile([128, BATCH * S], dt.bfloat16, tag="rhs")
                            if nh > 1:
                                hx_v = hx[:].rearrange("p (b f) -> p b f", b=BATCH)[
                                    :, :nb, 0:feat].rearrange("p b (h c) -> p b h c", h=nh)
                                w8_v = w8[:, :nb * nh].rearrange(
                                    "p (b h) -> p b h", b=nb)[:, :, :, None].to_broadcast(
                                    [128, nb, nh, feat // nh])
                                rhs_v = rhs[:].rearrange("p (b f) -> p b f", b=BATCH)[
                                    :, :nb, 0:feat].rearrange("p b (h c) -> p b h c", h=nh)
                            else:
                                hx_v = hx[:].rearrange("p (b f) -> p b f", b=BATCH)[:, :nb, 0:feat]
                                w8_v = w8[:, :nb * nh].rearrange(
                                    "p (b h) -> p b h", b=nb).to_broadcast([128, nb, feat])
                                rhs_v = rhs[:].rearrange("p (b f) -> p b f", b=BATCH)[:, :nb, 0:feat]
                            nc.vector.tensor_tensor(out=rhs_v, in0=hx_v, in1=w8_v, op=OP.mult)
                            nc.vector.tensor_copy(
                                out=rhs[:].rearrange("p (b f) -> p b f", b=BATCH)[:, :nb, feat:S],
                                in_=w8[:, :nb * nh].rearrange("p (b h) -> p b h", b=nb))
                            for b in range(nb):
                                nc.tensor.matmul(
                                    out=psW[:], lhsT=Ot[:, b * 128:(b + 1) * 128],
                                    rhs=rhs[:, b * S:(b + 1) * S],
                                    start=first and b == 0,
                                    stop=(j + nb >= ng) and b == nb - 1)
                            first = False
                            j += nb
                        flush(w, psW, sb, p2)

            def flush1(w, psW, sb, p2):
                den = sb.tile([128, HEADS], dt.float32, tag="den")
                nc.vector.tensor_scalar(out=den[:], in0=psW[:, C1:C1 + HEADS],
                                        scalar1=1e-16, scalar2=None, op0=OP.add)
                recip = sb.tile([128, HEADS], dt.float32, tag="recip")
                nc.vector.reciprocal(out=recip[:], in_=den[:])
                A = sb.tile([128, C1], dt.bfloat16, tag="A")
                nc.vector.tensor_tensor(
                    out=A[:].rearrange("p (h c) -> p h c", h=HEADS),
                    in0=psW[:, 0:C1].rearrange("p (h c) -> p h c", h=HEADS),
                    in1=recip[:][:, :, None].to_broadcast([128, HEADS, HID]),
                    op=OP.mult)
                nc.vector.tensor_tensor(out=A[:], in0=A[:], in1=b1_t[:], op=OP.add)
                nc.scalar.activation(out=A[:], in_=A[:], func=AF.Relu)
                psT = p2.tile([128, 128], dt.bfloat16, tag="psT")
                nc.tensor.transpose(out=psT[:], in_=A[:], identity=ident_t[:])
                at = sb.tile([128, 128], dt.bfloat16, tag="at")
                nc.scalar.copy(out=at[:], in_=psT[:])
                ps2 = p2.tile([128, ROW2], dt.float32, tag="ps2")
                nc.tensor.matmul(out=ps2[:], lhsT=at[:], rhs=w2e_t[:], start=True, stop=True)
                h2sb = sb.tile([128, ROW2], dt.bfloat16, tag="h2sb")
                nc.vector.tensor_copy(out=h2sb[:], in_=ps2[:])
                rows = 128 if w < NWIN - 1 else LAST_ROWS
                nc.sync.dma_start(out=h2own[w * 128:w * 128 + rows, :], in_=h2sb[:rows, :])

            edge_layer(hext1[:, :], C1, HEADS,
                       lambda w: (adstloc[w * 128:(w + 1) * 128, :], 128), flush1)
            tc.strict_bb_all_engine_barrier()

            nc.gpsimd.collective_compute(
                "AllGather", OP.bypass,
                replica_groups=[list(range(NC))],
                ins=[h2own[:, :]], outs=[h2full[:, :]])
            tc.strict_bb_all_engine_barrier()

            # ---------------- Phase C: layer-2 edge aggregation --------------
            def flush2(w, psW, sb, p2):
                den = sb.tile([128, 1], dt.float32, tag="den2")
                nc.vector.tensor_scalar(out=den[:], in0=psW[:, OUT:OUT + 1],
                                        scalar1=1e-16, scalar2=None, op0=OP.add)
                recip = sb.tile([128, 1], dt.float32, tag="recip2")
                nc.vector.reciprocal(out=recip[:], in_=den[:])
                o2 = sb.tile([128, OUT], dt.float32, tag="o2")
                nc.vector.tensor_tensor(
                    out=o2[:], in0=psW[:, 0:OUT],
                    in1=recip[:][:, 0:1].to_broadcast([128, OUT]), op=OP.mult)
                nc.vector.tensor_tensor(out=o2[:], in0=o2[:], in1=b2_t[:], op=OP.add)
                eo = sb.tile([128, OUT], dt.float32, tag="eo")
                ssum = sb.tile([128, 1], dt.float32, tag="ssum")
                nc.scalar.activation(out=eo[:], in_=o2[:], func=AF.Exp, accum_out=ssum[:])
                lns = sb.tile([128, 1], dt.float32, tag="lns")
                nc.scalar.activation(out=lns[:], in_=ssum[:], func=AF.Ln)
                ls = sb.tile([128, OUT], dt.float32, tag="ls")
                nc.vector.tensor_scalar(out=ls[:], in0=o2[:], scalar1=lns[:, 0:1],
                                        scalar2=None, op0=OP.subtract)
                rows = 128 if w < NWIN - 1 else LAST_ROWS
                nc.sync.dma_start(out=out[w * 128:w * 128 + rows, :], in_=ls[:rows, :])

            edge_layer(h2full[:, :], OUT, 1,
                       lambda w: (h2own[w * 128:min((w + 1) * 128, NLOC), 65:66],
                                  128 if w < NWIN - 1 else LAST_ROWS), flush2)

    nc.finalize()
    return nc


_CACHE = {}


def _prepare(x, edge_index, W1, att_src1, att_dst1, bias1, W2, att_src2,
             att_dst2, bias2):
    import ml_dtypes
    import concourse.bass  # noqa: F401  (ensures env boot)

    bf16 = ml_dtypes.bfloat16
    x = np.asarray(x, np.float32)
    xT = np.ascontiguousarray(x.T).astype(bf16)                 # [128, N]
    W1ext, W2ext = _pack_weights(W1, att_src1, att_dst1, W2, att_src2, att_dst2)
    b1_2d = np.broadcast_to(np.asarray(bias1, np.float32)[None, :], (128, C1)).copy()
    b2_2d = np.broadcast_to(np.asarray(bias2, np.float32)[None, :], (128, OUT)).copy()
    iota2d = np.broadcast_to(np.arange(128, dtype=np.float32)[None, :],
                             (128, 128)).astype(bf16).copy()
    ident = np.eye(128, dtype=np.float32).astype(bf16)

    srcsT, dstwT, ngroups, G = _preprocess(np.asarray(edge_index))
    xownT_all = np.zeros((NC, IN, NWIN * 128), np.float32)
    for k in range(NC):
        xownT_all[k, :, :NLOC] = np.asarray(x.T)[:, k * NLOC:(k + 1) * NLOC]
    xownT_all = xownT_all.astype(bf16)

    key = ("nc", G, tuple(ngroups))
    if key not in _CACHE:
        _CACHE[key] = _build_nc(ngroups, G)
    nc = _CACHE[key]

    in_maps = []
    for k in range(NC):
        in_maps.append({
            "xT": xT, "W1e": W1ext, "W2e": W2ext, "iota2d": iota2d,
            "ident": ident, "b1_2d": b1_2d, "b2_2d": b2_2d,
            "srcsT": srcsT[k], "dstwT": dstwT[k], "xownT": xownT_all[k],
        })
    return nc, in_maps


def kernel(x, edge_index, W1, att_src1, att_dst1, bias1, W2, att_src2, att_dst2, bias2):
    from concourse.bass_utils import run_bass_kernel_spmd
    nc, in_maps = _prepare(x, edge_index, W1, att_src1, att_dst1, bias1,
                           W2, att_src2, att_dst2, bias2)
    res = run_bass_kernel_spmd(nc, in_maps, list(range(NC)))
    return np.concatenate([res.results[k]["out"] for k in range(NC)], axis=0)



# revision 33
# speedup vs baseline: 1.4527x; 1.4527x over previous
"""Two-layer GAT on 8 Trainium2 NeuronCores — old gather structure + bf16.

Same design as the original baseline (dense dst-window edge columns, int32
indirect gathers, one-hot scatter with PE-transpose a_dst expansion) but with
bf16 tables, gathers, one-hots, and matmuls: ~2x less gather traffic and ~4x
faster PE ops. Scores/softmax stay fp32. Epsilon added to softmax denominators
(reference does the same) so empty pad rows yield finite garbage instead of
NaN that poisons layer-2 matmuls.
"""

import numpy as np

# ---- problem constants (hardcoded per harness contract) ----
N = 100000
E = 1600000
IN = 128
HID = 16
HEADS = 8
OUT = 64
NEG = 0.2
NC = 8
NLOC = N // NC          # 12500
WIN = 128
NWIN = (NLOC + WIN - 1) // WIN   # 98
LAST_ROWS = NLOC - (NWIN - 1) * WIN  # 84
C1 = HEADS * HID        # 128
ROW1 = C1 + 2 * HEADS   # 144 = z(128) | a_src(8) | a_dst(8)
ROW2 = OUT + 2          # 66  = h2(64) | a_src2(1) | a_dst2(1)
BATCH = 8               # edge groups per batched compute slab
PHA_B = 3               # Phase A node tiles per psum bank (3*144=432 <= 512)


def _preprocess(edge_index):
    """Per-core edge arrays in partition-major layout + shared group counts."""
    src = np.concatenate([np.asarray(edge_index[0]), np.arange(N)]).astype(np.int64)
    dst = np.concatenate([np.asarray(edge_index[1]), np.arange(N)]).astype(np.int64)
    core = dst // NLOC
    per_core = []
    cnts = np.zeros((NC, NWIN), dtype=np.int64)
    for k in range(NC):
        m = core == k
        s, d = src[m], dst[m] - k * NLOC
        o = np.argsort(d, kind="stable")
        s, d = s[o], d[o]
        per_core.append((s, d))
        cnts[k] = np.bincount(d // WIN, minlength=NWIN)
    ngroups = np.maximum(1, ((cnts + 127) // 128).max(axis=0))  # shared, >=1
    G = int(ngroups.sum())
    gstart = np.concatenate([[0], np.cumsum(ngroups)])
    srcsT = np.zeros((NC, 128, G), dtype=np.int32)
    dstwT = np.full((NC, 128, G), 999.0, dtype=np.float32)
    for k in range(NC):
        s, d = per_core[k]
        w = d // WIN
        ws = np.searchsorted(w, np.arange(NWIN))
        we = np.searchsorted(w, np.arange(NWIN), side="right")
        fs = np.zeros(G * 128, dtype=np.int64)
        fw = np.full(G * 128, 999.0, dtype=np.float32)
        for wi in range(NWIN):
            cnt = we[wi] - ws[wi]
            a = gstart[wi] * 128
            fs[a:a + cnt] = s[ws[wi]:we[wi]]
            fw[a:a + cnt] = (d[ws[wi]:we[wi]] - wi * WIN).astype(np.float32)
        srcsT[k] = fs.reshape(G, 128).T.astype(np.int32)
        dstwT[k] = fw.reshape(G, 128).T
    return srcsT, dstwT, ngroups.tolist(), G


def _pack_weights(W1, att_src1, att_dst1, W2, att_src2, att_dst2):
    import ml_dtypes
    W1 = np.asarray(W1, np.float32)
    W2 = np.asarray(W2, np.float32)
    A1s = np.zeros((C1, HEADS), np.float32)
    A1d = np.zeros((C1, HEADS), np.float32)
    for h in range(HEADS):
        A1s[h * HID:(h + 1) * HID, h] = np.asarray(att_src1, np.float32)[h]
        A1d[h * HID:(h + 1) * HID, h] = np.asarray(att_dst1, np.float32)[h]
    W1ext = np.concatenate([W1, W1 @ A1s, W1 @ A1d], axis=1)   # [128, 144]
    W2ext = np.concatenate(
        [W2, W2 @ np.asarray(att_src2, np.float32).T,
         W2 @ np.asarray(att_dst2, np.float32).T], axis=1)     # [128, 66]
    return (np.ascontiguousarray(W1ext).astype(ml_dtypes.bfloat16),
            np.ascontiguousarray(W2ext).astype(ml_dtypes.bfloat16))


def _build_nc(ngroups, G):
    import concourse.bass as bass
    import concourse.bacc as bacc
    import concourse.mybir as mybir
    import concourse.tile as tile

    dt = mybir.dt
    AF = mybir.ActivationFunctionType
    OP = mybir.AluOpType
    nc = bacc.Bacc("TRN2", target_bir_lowering=False, debug=False, num_devices=NC)

    xT = nc.dram_tensor("xT", [IN, N], dt.bfloat16, kind="ExternalInput")
    W1e = nc.dram_tensor("W1e", [IN, ROW1], dt.bfloat16, kind="ExternalInput")
    W2e = nc.dram_tensor("W2e", [C1, ROW2], dt.bfloat16, kind="ExternalInput")
    iota2d = nc.dram_tensor("iota2d", [128, 128], dt.bfloat16, kind="ExternalInput")
    ident = nc.dram_tensor("ident", [128, 128], dt.bfloat16, kind="ExternalInput")
    b1_2d = nc.dram_tensor("b1_2d", [128, C1], dt.float32, kind="ExternalInput")
    b2_2d = nc.dram_tensor("b2_2d", [128, OUT], dt.float32, kind="ExternalInput")
    srcsT = nc.dram_tensor("srcsT", [128, G], dt.int32, kind="ExternalInput")
    dstwT = nc.dram_tensor("dstwT", [128, G], dt.float32, kind="ExternalInput")
    xownT = nc.dram_tensor("xownT", [IN, NWIN * 128], dt.bfloat16, kind="ExternalInput")
    out = nc.dram_tensor("out", [NLOC, OUT], dt.float32, kind="ExternalOutput")

    hext1 = nc.dram_tensor("hext1", [N, ROW1], dt.bfloat16)
    adstloc = nc.dram_tensor("adstloc", [NWIN * 128, HEADS], dt.bfloat16)
    h2own = nc.dram_tensor("h2own", [NLOC, ROW2], dt.bfloat16)
    h2full = nc.dram_tensor("h2full", [N, ROW2], dt.bfloat16, addr_space="Shared")

    gstart = np.concatenate([[0], np.cumsum(ngroups)]).astype(int)

    with tile.TileContext(nc) as tc:
        with tc.tile_pool(name="const", bufs=1) as cb:
            w1e_t = cb.tile([IN, ROW1], dt.bfloat16)
            nc.sync.dma_start(out=w1e_t[:], in_=W1e[:, :])
            w2e_t = cb.tile([C1, ROW2], dt.bfloat16)
            nc.sync.dma_start(out=w2e_t[:], in_=W2e[:, :])
            iota_t = cb.tile([128, 128], dt.bfloat16)
            nc.sync.dma_start(out=iota_t[:], in_=iota2d[:, :])
            ident_t = cb.tile([128, 128], dt.bfloat16)
            nc.sync.dma_start(out=ident_t[:], in_=ident[:, :])
            b1_t = cb.tile([128, C1], dt.float32)
            nc.sync.dma_start(out=b1_t[:], in_=b1_2d[:, :])
            b2_t = cb.tile([128, OUT], dt.float32)
            nc.sync.dma_start(out=b2_t[:], in_=b2_2d[:, :])
            tc.strict_bb_all_engine_barrier()

            # ---------------- Phase A: hext1 = x @ W1ext (replicated) --------
            with (
                tc.tile_pool(name="pha_sb", bufs=3) as sa,
                tc.tile_pool(name="pha_ps", bufs=2, space="PSUM") as pa,
            ):
                ntile = (N + 127) // 128  # 782, last has 32 rows
                t = 0
                while t < ntile:
                    nb = min(PHA_B, ntile - t)
                    r0 = t * 128
                    rows = min(nb * 128, N - r0)
                    xt = sa.tile([IN, nb * 128], dt.bfloat16, tag="xt")
                    nc.sync.dma_start(out=xt[:, :rows], in_=xT[:, r0:r0 + rows])
                    psA = pa.tile([128, nb * ROW1], dt.float32, tag="psA")
                    for b in range(nb):
                        rr = min(128, N - (t + b) * 128)
                        nc.tensor.matmul(
                            out=psA[:rr, b * ROW1:(b + 1) * ROW1],
                            lhsT=xt[:, b * 128:b * 128 + rr],
                            rhs=w1e_t[:], start=True, stop=True)
                    zs = sa.tile([128, nb * ROW1], dt.bfloat16, tag="zs")
                    if rows % 128 == 0:
                        nc.vector.tensor_copy(out=zs[:], in_=psA[:])
                        nc.sync.dma_start(
                            out=hext1[r0:r0 + rows, :].rearrange(
                                "(b p) f -> p b f", p=128),
                            in_=zs[:].rearrange("p (b f) -> p b f", b=nb))
                    else:
                        for b in range(nb):
                            rr = min(128, N - (t + b) * 128)
                            nc.vector.tensor_copy(
                                out=zs[:rr, b * ROW1:(b + 1) * ROW1],
                                in_=psA[:rr, b * ROW1:(b + 1) * ROW1])
                            nc.sync.dma_start(
                                out=hext1[(t + b) * 128:(t + b) * 128 + rr, :],
                                in_=zs[:rr, b * ROW1:(b + 1) * ROW1])
                    t += nb
                # Phase A2: own-shard a_dst table (window-padded, core-local)
                for w in range(NWIN):
                    xo = sa.tile([IN, 128], dt.bfloat16, tag="xo")
                    nc.sync.dma_start(out=xo[:], in_=xownT[:, w * 128:(w + 1) * 128])
                    psA2 = pa.tile([128, HEADS], dt.float32, tag="psA2")
                    nc.tensor.matmul(out=psA2[:], lhsT=xo[:],
                                     rhs=w1e_t[:, C1 + HEADS:ROW1], start=True, stop=True)
                    a2s = sa.tile([128, HEADS], dt.bfloat16, tag="a2s")
                    nc.vector.tensor_copy(out=a2s[:], in_=psA2[:])
                    nc.sync.dma_start(out=adstloc[w * 128:(w + 1) * 128, :], in_=a2s[:])
            tc.strict_bb_all_engine_barrier()

            # ---------------- edge aggregation pipeline ----------------------
            def edge_layer(table_ap, feat, nh, adw_src, flush):
                S = feat + nh
                with (
                    tc.tile_pool(name="eb_sb", bufs=3) as sb,
                    tc.tile_pool(name="eb_idx", bufs=2) as sx,
                    tc.tile_pool(name="eb_ps", bufs=2, space="PSUM") as pw,
                    tc.tile_pool(name="eb_pot", bufs=2, space="PSUM") as pot,
                    tc.tile_pool(name="eb_pad", bufs=2, space="PSUM") as pad,
                    tc.tile_pool(name="eb_ps2", bufs=1, space="PSUM") as p2,
                ):
                    for w in range(NWIN):
                        g0, g1 = int(gstart[w]), int(gstart[w + 1])
                        ng = g1 - g0
                        src_t = sx.tile([128, ng], dt.int32, tag="src")
                        dw_t = sx.tile([128, ng], dt.float32, tag="dw")
                        nc.sync.dma_start(out=src_t[:], in_=srcsT[:, g0:g1])
                        nc.sync.dma_start(out=dw_t[:], in_=dstwT[:, g0:g1])
                        adw_ap, adw_rows = adw_src(w)
                        adw_t = sx.tile([128, nh], dt.bfloat16, tag="adw")
                        if adw_rows < 128:
                            nc.gpsimd.memset(adw_t[:], 0.0)
                        nc.sync.dma_start(out=adw_t[:adw_rows, :], in_=adw_ap)
                        psW = pw.tile([128, S], dt.float32, tag="psW")
                        j = 0
                        first = True
                        while j < ng:
                            nb = min(BATCH, ng - j)
                            hx = sb.tile([128, BATCH * S], dt.bfloat16, tag="hx")
                            ad = sb.tile([128, BATCH * nh], dt.float32, tag="ad")
                            for b in range(nb):
                                nc.gpsimd.indirect_dma_start(
                                    out=hx[:, b * S:(b + 1) * S],
                                    out_offset=None, in_=table_ap,
                                    in_offset=bass.IndirectOffsetOnAxis(
                                        ap=src_t[:, j + b:j + b + 1], axis=0))
                            Ot = sb.tile([128, BATCH * 128], dt.bfloat16, tag="Ot")
                            for b in range(nb):
                                nc.vector.tensor_scalar(
                                    out=Ot[:, b * 128:(b + 1) * 128], in0=iota_t[:],
                                    scalar1=dw_t[:, j + b:j + b + 1], scalar2=None,
                                    op0=OP.is_equal)
                            # a_dst_e = O @ a_dstW  (transpose O on PE, then matmul)
                            for b in range(nb):
                                psOT = pot.tile([128, 128], dt.bfloat16, tag="psOT")
                                nc.tensor.transpose(
                                    out=psOT[:], in_=Ot[:, b * 128:(b + 1) * 128],
                                    identity=ident_t[:])
                                ot_sb = sb.tile([128, 128], dt.bfloat16, tag="otsb")
                                nc.scalar.copy(out=ot_sb[:], in_=psOT[:])
                                psAD = pad.tile([128, nh], dt.float32, tag="psAD")
                                nc.tensor.matmul(out=psAD[:], lhsT=ot_sb[:],
                                                 rhs=adw_t[:], start=True, stop=True)
                                nc.scalar.copy(out=ad[:, b * nh:(b + 1) * nh], in_=psAD[:])
                            # e = a_src + a_dst ; w = exp(max(e, 0.2e))
                            ev = sb.tile([128, BATCH * nh], dt.float32, tag="ev")
                            asrc_v = hx[:].rearrange("p (b f) -> p b f", b=BATCH)[:, :nb, feat:S]
                            nc.vector.tensor_tensor(
                                out=ev[:, :nb * nh].rearrange("p (b h) -> p b h", b=nb),
                                in0=asrc_v, in1=ad[:, :nb * nh].rearrange(
                                    "p (b h) -> p b h", b=nb), op=OP.add)
                            sc = sb.tile([128, BATCH * nh], dt.float32, tag="sc")
                            nc.scalar.mul(out=sc[:, :nb * nh], in_=ev[:, :nb * nh], mul=NEG)
                            w8 = sb.tile([128, BATCH * nh], dt.float32, tag="w8")
                            nc.vector.tensor_tensor(out=w8[:, :nb * nh], in0=ev[:, :nb * nh],
                                                    in1=sc[:, :nb * nh], op=OP.max)
                            nc.scalar.activation(out=w8[:, :nb * nh], in_=w8[:, :nb * nh],
                                                 func=AF.Exp)
                            # weighted rhs
                            rhs = sb.tile([128, BATCH * S], dt.bfloat16, tag="rhs")
                            if nh > 1:
                                hx_v = hx[:].rearrange("p (b f) -> p b f", b=BATCH)[
                                    :, :nb, 0:feat].rearrange("p b (h c) -> p b h c", h=nh)
                                w8_v = w8[:, :nb * nh].rearrange(
                                    "p (b h) -> p b h", b=nb)[:, :, :, None].to_broadcast(
                                    [128, nb, nh, feat // nh])
                                rhs_v = rhs[:].rearrange("p (b f) -> p b f", b=BATCH)[
                                    :, :nb, 0:feat].rearrange("p b (h c) -> p b h c", h=nh)
                            else:
                                hx_v = hx[:].rearrange("p (b f) -> p b f", b=BATCH)[:, :nb, 0:feat]
                                w8_v = w8[:, :nb * nh].rearrange(
                                    "p (b h) -> p b h", b=nb).to_broadcast([128, nb, feat])
                                rhs_v = rhs[:].rearrange("p (b f) -> p b f", b=BATCH)[:, :nb, 0:feat]
                            nc.vector.tensor_tensor(out=rhs_v, in0=hx_v, in1=w8_v, op=OP.mult)
                            nc.vector.tensor_copy(
                                out=rhs[:].rearrange("p (b f) -> p b f", b=BATCH)[:, :nb, feat:S],
                                in_=w8[:, :nb * nh].rearrange("p (b h) -> p b h", b=nb))
                            for b in range(nb):
                                nc.tensor.matmul(
                                    out=psW[:], lhsT=Ot[:, b * 128:(b + 1) * 128],
                                    rhs=rhs[:, b * S:(b + 1) * S],
                                    start=first and b == 0,
                                    stop=(j + nb >= ng) and b == nb - 1)
                            first = False
                            j += nb
                        flush(w, psW, sb, p2)

            def flush1(w, psW, sb, p2):
                den = sb.tile([128, HEADS], dt.float32, tag="den")
                nc.vector.tensor_scalar(out=den[:], in0=psW[:, C1:C1 + HEADS],
                                        scalar1=1e-16, scalar2=None, op0=OP.add)
                recip = sb.tile([128, HEADS], dt.float32, tag="recip")
                nc.vector.reciprocal(out=recip[:], in_=den[:])
                A = sb.tile([128, C1], dt.bfloat16, tag="A")
                nc.vector.tensor_tensor(
                    out=A[:].rearrange("p (h c) -> p h c", h=HEADS),
                    in0=psW[:, 0:C1].rearrange("p (h c) -> p h c", h=HEADS),
                    in1=recip[:][:, :, None].to_broadcast([128, HEADS, HID]),
                    op=OP.mult)
                nc.vector.tensor_tensor(out=A[:], in0=A[:], in1=b1_t[:], op=OP.add)
                nc.scalar.activation(out=A[:], in_=A[:], func=AF.Relu)
                psT = p2.tile([128, 128], dt.bfloat16, tag="psT")
                nc.tensor.transpose(out=psT[:], in_=A[:], identity=ident_t[:])
                at = sb.tile([128, 128], dt.bfloat16, tag="at")
                nc.scalar.copy(out=at[:], in_=psT[:])
                ps2 = p2.tile([128, ROW2], dt.float32, tag="ps2")
                nc.tensor.matmul(out=ps2[:], lhsT=at[:], rhs=w2e_t[:], start=True, stop=True)
                h2sb = sb.tile([128, ROW2], dt.bfloat16, tag="h2sb")
                nc.vector.tensor_copy(out=h2sb[:], in_=ps2[:])
                rows = 128 if w < NWIN - 1 else LAST_ROWS
                nc.sync.dma_start(out=h2own[w * 128:w * 128 + rows, :], in_=h2sb[:rows, :])

            edge_layer(hext1[:, :], C1, HEADS,
                       lambda w: (adstloc[w * 128:(w + 1) * 128, :], 128), flush1)
            tc.strict_bb_all_engine_barrier()

            nc.gpsimd.collective_compute(
                "AllGather", OP.bypass,
                replica_groups=[list(range(NC))],
                ins=[h2own[:, :]], outs=[h2full[:, :]])
            tc.strict_bb_all_engine_barrier()

            # ---------------- Phase C: layer-2 edge aggregation --------------
            def flush2(w, psW, sb, p2):
                den = sb.tile([128, 1], dt.float32, tag="den2")
                nc.vector.tensor_scalar(out=den[:], in0=psW[:, OUT:OUT + 1],
                                        scalar1=1e-16, scalar2=None, op0=OP.add)
                recip = sb.tile([128, 1], dt.float32, tag="recip2")
                nc.vector.reciprocal(out=recip[:], in_=den[:])
                o2 = sb.tile([128, OUT], dt.float32, tag="o2")
                nc.vector.tensor_tensor(
                    out=o2[:], in0=psW[:, 0:OUT],
                    in1=recip[:][:, 0:1].to_broadcast([128, OUT]), op=OP.mult)
                nc.vector.tensor_tensor(out=o2[:], in0=o2[:], in1=b2_t[:], op=OP.add)
                eo = sb.tile([128, OUT], dt.float32, tag="eo")
                ssum = sb.tile([128, 1], dt.float32, tag="ssum")
                nc.scalar.activation(out=eo[:], in_=o2[:], func=AF.Exp, accum_out=ssum[:])
                lns = sb.tile([128, 1], dt.float32, tag="lns")
                nc.scalar.activation(out=lns[:], in_=ssum[:], func=AF.Ln)
                ls = sb.tile([128, OUT], dt.float32, tag="ls")
                nc.vector.tensor_scalar(out=ls[:], in0=o2[:], scalar1=lns[:, 0:1],
                                        scalar2=None, op0=OP.subtract)
                rows = 128 if w < NWIN - 1 else LAST_ROWS
                nc.sync.dma_start(out=out[w * 128:w * 128 + rows, :], in_=ls[:rows, :])

            edge_layer(h2full[:, :], OUT, 1,
                       lambda w: (h2own[w * 128:min((w + 1) * 128, NLOC), 65:66],
                                  128 if w < NWIN - 1 else LAST_ROWS), flush2)

    nc.finalize()
    return nc


_CACHE = {}


def _prepare(x, edge_index, W1, att_src1, att_dst1, bias1, W2, att_src2,
             att_dst2, bias2):
    import ml_dtypes
    import concourse.bass  # noqa: F401  (ensures env boot)

    bf16 = ml_dtypes.bfloat16
    x = np.asarray(x, np.float32)
    xT = np.ascontiguousarray(x.T).astype(bf16)                 # [128, N]
    W1ext, W2ext = _pack_weights(W1, att_src1, att_dst1, W2, att_src2, att_dst2)
    b1_2d = np.broadcast_to(np.asarray(bias1, np.float32)[None, :], (128, C1)).copy()
    b2_2d = np.broadcast_to(np.asarray(bias2, np.float32)[None, :], (128, OUT)).copy()
    iota2d = np.broadcast_to(np.arange(128, dtype=np.float32)[None, :],
                             (128, 128)).astype(bf16).copy()
    ident = np.eye(128, dtype=np.float32).astype(bf16)

    srcsT, dstwT, ngroups, G = _preprocess(np.asarray(edge_index))
    xownT_all = np.zeros((NC, IN, NWIN * 128), np.float32)
    for k in range(NC):
        xownT_all[k, :, :NLOC] = np.asarray(x.T)[:, k * NLOC:(k + 1) * NLOC]
    xownT_all = xownT_all.astype(bf16)

    key = ("nc", G, tuple(ngroups))
    if key not in _CACHE:
        _CACHE[key] = _build_nc(ngroups, G)
    nc = _CACHE[key]

    in_maps = []
    for k in range(NC):
        in_maps.append({
            "xT": xT, "W1e": W1ext, "W2e": W2ext, "iota2d": iota2d,
            "ident": ident, "b1_2d": b1_2d, "b2_2d": b2_2d,
            "srcsT": srcsT[k], "dstwT": dstwT[k], "xownT": xownT_all[k],
        })
    return nc, in_maps


def kernel(x, edge_index, W1, att_src1, att_dst1, bias1, W2, att_src2, att_dst2, bias2):
    from concourse.bass_utils import run_bass_kernel_spmd
    nc, in_maps = _prepare(x, edge_index, W1, att_src1, att_dst1, bias1,
                           W2, att_src2, att_dst2, bias2)
    res = run_bass_kernel_spmd(nc, in_maps, list(range(NC)))
    return np.concatenate([res.results[k]["out"] for k in range(NC)], axis=0)

